# revision 1
# baseline (speedup 1.0000x reference)
"""2-layer GCN (EvolveGCN eval forward) on 8 Trainium2 NeuronCores.

Strategy (graph/data parallel, dst-sharded, per sharding hint):
  - Nodes partitioned contiguously across 8 cores (6250 each, 49 blocks of
    128). Each core owns the edges whose dst lands in its partition and
    produces the output rows for its partition.
  - Math: with D = diag(1/sqrt(deg+1)),
      out1 = relu(LN(D [A^ (D X W1)])) ; out2 = relu(LN(D [A^ (D out1 W2)]))
    where A^ includes self loops. LN is row-scale invariant, so the dst-side
    D drops out entirely; the src-side D is folded into the operands:
      L1: host pre-gathers rows of Xs = D*X per edge into a *sequential*
          stream; aggregation (one-hot S matmuls on PE) runs BEFORE W1,
          since (sum Xs[src]) @ W1 == sum (Xs W1)[src].
      L2: s1 = D*out1 is produced by the L1 epilogue; hs2 = s1 @ W2 rows are
          AllGathered (the only collective) and per-edge rows fetched with
          the GPSIMD dma_gather custom DMA (int16 indices, two 25088-row
          windows).
  - Edge slots are packed continuously (runs may span dst-block boundaries);
    per-(block[,half]) sizes are maxed over cores so the compiled program is
    uniform (SPMD) while per-core data (stream rows, indices, dst-row keys)
    varies. One-hot S matrices are built via tensor_scalar is_equal with a
    per-partition f32 scalar (hits the DVE 2x perf mode; faster than
    broadcast tensor_tensor); in layer 1 ~30% of the builds run on the
    otherwise-idle GPSIMD/Pool engine (InstTensorScalarPtr is a built-in,
    no library switch needed next to dma_gather's mlp library).
  - Epilogue: bn_stats/bn_aggr (DVE) for LN stats, sqrt batched across a
    block group on ACT, single fused scale+bias+relu ACT op per block;
    PSUM->SBUF copies ride the ACT engine.

Host-side work is graph preprocessing and input layout (integer bucketing,
index building, gathers/casts of input rows, one elementwise dinv scaling of
X); all matmuls, aggregation, LN and activations run on device.
"""
import os as _os
import sys

sys.path.insert(0, "/opt/trn_rl_repo")

import numpy as np
import ml_dtypes

import concourse.bacc as bacc
import concourse.bass as bass
import concourse.mybir as mybir
import concourse.tile as tile
from concourse.tile import add_dep_helper
from concourse.bass_utils import run_bass_kernel_spmd
from concourse.library_config import mlp as _mlp_lib
from concourse._compat import cdiv

P = 128
NCORES = 8
N_NODES = 50000
PART = N_NODES // NCORES          # 6250 real nodes per core
NB = cdiv(PART, P)                # 49 dst blocks per core
PPAD = NB * P                     # 6272 padded nodes per core
NPAD = NCORES * PPAD              # 50176 padded global nodes
SPLIT = NPAD // 2                 # 25088 (< 32768 so int16 indices work)
F = 128                           # feature dim
LN_EPS = 1e-5
GRP = int(_os.environ.get("K_GRP", "4"))
L1_CHUNK = int(_os.environ.get("K_L1_CHUNK", "32"))
L2_CHUNK = int(_os.environ.get("K_L2_CHUNK", "56"))
GPOOL_BUFS = int(_os.environ.get("K_GPOOL", "7"))
N_GSEMS = 8
COPY_ENGINE = _os.environ.get("K_COPY_ENGINE", "act")
# percent of S one-hot tiles host-streamed (vs DVE-built), per layer
S_FRAC = (int(_os.environ.get("K_S1_FRAC", "0")), int(_os.environ.get("K_S2_FRAC", "0")))
SS_CHUNK = 24                     # streamed-S tiles per DMA chunk


def _s_streamed(l, op):
    return (op % 100) < S_FRAC[l]


SBATCH = (int(_os.environ.get("K_SBATCH1", "0")), int(_os.environ.get("K_SBATCH2", "0")))
SBATCH_MAX = 20
SACT = (int(_os.environ.get("K_SACT1", "0")), int(_os.environ.get("K_SACT2", "0")))
SPOOL = (int(_os.environ.get("K_SPOOL1", "30")), int(_os.environ.get("K_SPOOL2", "0")))


def _s_on_act(l, op):
    f = SACT[l]
    return f > 0 and (op * f) % 100 < f


def _s_on_pool(l, op):
    f = SPOOL[l]
    return f > 0 and (op * f) % 100 < f

bf16 = ml_dtypes.bfloat16


# ---------------------------------------------------------------- host prep
def _wrap16(arr):
    """[L] int -> [128, L/16] int16, index j of the stream at [j%16, j//16],
    replicated to all 8 GPSIMD core partition groups."""
    L = arr.shape[0]
    assert L % 16 == 0
    a2 = arr.reshape(L // 16, 16).T          # [16, S]
    return np.ascontiguousarray(np.tile(a2, (8, 1)).astype(np.int16))


def _l2_op_order(rs2, re2, b):
    """Per-block L2 op order. K_L2IL=1 interleaves the A/B gather streams
    run-by-run; default keeps all-A-then-all-B."""
    A = [(0, r) for r in range(rs2[0][b], re2[0][b])]
    B = [(1, r) for r in range(rs2[1][b], re2[1][b])]
    if _os.environ.get("K_L2IL", "0") != "1":
        return A + B
    out = []
    for i in range(max(len(A), len(B))):
        if i < len(A):
            out.append(A[i])
        if i < len(B):
            out.append(B[i])
    return out


def _op_ranges(Off, Sz):
    """Run-index range [rs, re) touched by each block's slot interval."""
    rs = [int(Off[b]) // P for b in range(len(Sz))]
    re = [int(cdiv(Off[b] + Sz[b], P)) for b in range(len(Sz))]
    return rs, re


def _dr_ops(sdr, sblk, rs, re, nops):
    """Per-op dst-row columns: op (b, r) -> dr[p] = sdr[r*P+p] if that slot
    belongs to block b else -1. sdr/sblk are slot-level arrays."""
    out = np.full((nops, P), -1.0, dtype=np.float32)
    col = 0
    sdr2 = sdr.reshape(-1, P)
    sblk2 = sblk.reshape(-1, P)
    for b in range(len(rs)):
        for r in range(rs[b], re[b]):
            m = sblk2[r] == b
            out[col, m] = sdr2[r, m]
            col += 1
    assert col == nops
    return np.ascontiguousarray(out.T)       # [P, nops] f32


def _prep(x, edge_index):
    """Graph preprocessing + per-core input layout. Returns (in_maps, struct)
    where struct holds the uniform compile-time run structure."""
    src = np.asarray(edge_index[0], dtype=np.int64)
    dst = np.asarray(edge_index[1], dtype=np.int64)
    E = src.shape[0]

    counts = np.bincount(dst, minlength=N_NODES).astype(np.float32)
    dinv = (1.0 / np.sqrt(counts + 1.0)).astype(np.float32)
    Xs = (np.asarray(x, np.float32) * dinv[:, None]).astype(bf16)   # [N, F]

    core_of = dst // PART
    ldst = dst - core_of * PART              # 0..6249
    blk = ldst >> 7                          # 0..48
    drow = ldst & 127                        # dst row within block
    ps = (src // PART) * PPAD + (src % PART)  # padded global src id
    half = (ps >= SPLIT).astype(np.int64)

    # ---------------- L1 (streamed) structure: blocks only ----------------
    # (self loops enter via an identity matmul from the xself tile, not slots)
    cnt1 = np.bincount(core_of * NB + blk, minlength=NCORES * NB).reshape(NCORES, NB)
    Sz1 = cnt1.max(axis=0)
    Off1 = np.concatenate([[0], np.cumsum(Sz1)])
    T1 = int(Off1[-1])
    R1 = cdiv(T1, P)
    T1r = R1 * P
    rs1, re1 = _op_ranges(Off1, Sz1)
    n_ops1 = sum(re1[b] - rs1[b] for b in range(NB))

    # ---------------- L2 (gathered) structure: (half, block) --------------
    cnt2 = np.bincount((core_of * 2 + half) * NB + blk,
                       minlength=NCORES * 2 * NB).reshape(NCORES, 2, NB)
    Sz2 = cnt2.max(axis=0)                   # [2, NB]
    Off2, R2, rs2, re2 = [], [], [], []
    for h in range(2):
        off = np.concatenate([[0], np.cumsum(Sz2[h])])
        Off2.append(off)
        R2.append(cdiv(int(off[-1]), P))
        a, b_ = _op_ranges(off, Sz2[h])
        rs2.append(a)
        re2.append(b_)
    RA, RB = R2
    n_ops2 = sum(re2[h][b] - rs2[h][b] for h in range(2) for b in range(NB))

    # ---------------- per-core data ----------------
    order1 = np.lexsort((src, blk, core_of))
    order2 = np.lexsort((ps, blk, half, core_of))
    c1 = np.cumsum(cnt1.reshape(-1))         # segment ends in order1
    c2 = np.cumsum(cnt2.reshape(-1))         # segment ends in order2 (c,h,b)

    in_maps = []
    for c in range(NCORES):
        # ---- L1 slots
        g0idx = np.zeros(T1r, dtype=np.int64)
        sdr1 = np.zeros(T1r, dtype=np.float32)
        sblk1 = np.full(T1r, -1, dtype=np.int64)
        for b in range(NB):
            k = c * NB + b
            e0 = c1[k] - cnt1[c, b]
            n = cnt1[c, b]
            o = int(Off1[b])
            sel = order1[e0:e0 + n]
            g0idx[o:o + n] = src[sel]
            sdr1[o:o + n] = drow[sel]
            sblk1[o:o + n] = b
        G0 = np.ascontiguousarray(
            Xs[g0idx].reshape(R1, P, F).transpose(1, 0, 2))        # [P,R1,F]
        dr1 = _dr_ops(sdr1, sblk1, rs1, re1, n_ops1)

        # ---- L2 slots (per half)
        idx_w, drh = [], []
        for h in range(2):
            Th = int(Off2[h][-1])
            Thr = R2[h] * P
            sidx = np.zeros(max(Thr, 16), dtype=np.int64)
            sdr = np.zeros(max(Thr, P), dtype=np.float32)
            sblk = np.full(max(Thr, P), -1, dtype=np.int64)
            for b in range(NB):
                k = (c * 2 + h) * NB + b
                e0 = c2[k] - cnt2[c, h, b]
                n = cnt2[c, h, b]
                o = int(Off2[h][b])
                sel = order2[e0:e0 + n]
                sidx[o:o + n] = ps[sel] - h * SPLIT
                sdr[o:o + n] = drow[sel]
                sblk[o:o + n] = b
            idx_w.append(_wrap16(sidx[:max(Thr, 16)]) if Thr
                         else np.zeros((128, 1), np.int16))
            drh.append((sdr[:Thr], sblk[:Thr]))
        # dr2 columns in op order: for b: for h: for r
        dr2 = np.full((n_ops2, P), -1.0, dtype=np.float32)
        col = 0
        for b in range(NB):
            for h, r in _l2_op_order(rs2, re2, b):
                sdr, sblk = drh[h]
                sdr2_ = sdr.reshape(-1, P)
                sblk2_ = sblk.reshape(-1, P)
                m = sblk2_[r] == b
                dr2[col, m] = sdr2_[r, m]
                col += 1
        assert col == n_ops2
        dr2 = np.ascontiguousarray(dr2.T)

        cnt_loc = np.zeros(PPAD, dtype=np.float32)
        cnt_loc[:PART] = counts[c * PART:(c + 1) * PART]

        xs_loc = np.zeros((PPAD, F), dtype=bf16)
        xs_loc[:PART] = Xs[c * PART:(c + 1) * PART]
        xself = np.ascontiguousarray(
            xs_loc.reshape(NB, P, F).transpose(1, 0, 2))           # [P,NB,F]
        m = {
            "g0": G0,
            "xself": xself,
            "dr1": dr1,
            "dr2": dr2,
            "idxA": idx_w[0],
            "idxB": idx_w[1],
            "cnts": np.ascontiguousarray(cnt_loc.reshape(NB, P).T),  # [P,NB]
        }
        iot = np.arange(P, dtype=np.float32)
        for l, (drm, nops) in enumerate(((dr1, n_ops1), (dr2, n_ops2))):
            cols = [op for op in range(nops) if _s_streamed(l, op)]
            if cols:
                sel = drm[:, cols]                       # [P, ns]
                Sst = (sel[:, :, None] == iot[None, None, :]).astype(bf16)
            else:
                Sst = np.zeros((P, 1, P), dtype=bf16)
            m[f"s{l+1}s"] = np.ascontiguousarray(Sst)
        in_maps.append(m)

    ns1 = sum(1 for op in range(n_ops1) if _s_streamed(0, op))
    ns2 = sum(1 for op in range(n_ops2) if _s_streamed(1, op))
    struct = dict(R1=R1, RA=RA, RB=RB, rs1=rs1, re1=re1, rs2=rs2, re2=re2,
                  n_ops1=n_ops1, n_ops2=n_ops2, ns1=ns1, ns2=ns2)
    return in_maps, struct


# ---------------------------------------------------------------- device code
def _inst(x):
    import bass_rust as _br
    if isinstance(x, _br.Instruction):
        return x
    return x.ins


class _Gather:
    """Chunked dma_gather stream with software prefetch (from the baseline):
    descriptor generation / DMA of later chunks overlaps the wait on the
    current one. Consumers must register via dep()."""

    PREFETCH = int(_os.environ.get("K_GPF", "2"))

    def __init__(self, nc, tc, pool, sems, sem_counts, idx_t, src_ap, n_runs,
                 name, queue_num=0):
        self.nc, self.tc, self.pool = nc, tc, pool
        self.sems, self.sem_counts = sems, sem_counts
        self.idx_t, self.src_ap, self.n_runs = idx_t, src_ap, n_runs
        self.n_chunks = cdiv(n_runs, L2_CHUNK)
        self.name = name
        self.queue_num = queue_num
        self.issued = -1
        self.sem_i = 0
        self.tiles = {}
        self.wi = {}
        self.semtgt = {}
        self.dep_done = set()

    def _emit_issue(self, ci):
        nruns = min(L2_CHUNK, self.n_runs - ci * L2_CHUNK)
        n = nruns * P
        off = ci * L2_CHUNK * P
        g = self.pool.tile([P, L2_CHUNK, F], mybir.dt.bfloat16,
                           tag="gchunk", name=f"g_{self.name}_{ci}")
        k = self.sem_i % len(self.sems)
        self.sem_i += 1
        sem = self.sems[k]
        self.sem_counts[k] += 16
        self.semtgt[ci] = (sem, self.sem_counts[k])
        self.nc.gpsimd.dma_gather(
            g[:, :nruns, :], self.src_ap,
            self.idx_t[:, off // 16:(off + n) // 16],
            n, n, F, single_packet=(_os.environ.get('K_SP','0')=='1'), queue_num=self.queue_num,
        ).then_inc(sem, 16)
        self.tiles[ci] = g
        self.issued = ci

    def rhs(self, r):
        ci = r // L2_CHUNK
        if ci not in self.wi:
            with self.tc.tile_critical(no_gpsimd_drain=True):
                while self.issued < min(ci + self.PREFETCH, self.n_chunks - 1):
                    self._emit_issue(self.issued + 1)
                sem, tgt = self.semtgt[ci]
                self.nc.gpsimd.wait_ge(sem, tgt)
            self.wi[ci] = self.tc.prev_crit_insts[mybir.EngineType.Pool]
            self.tiles.pop(ci - 1, None)
        return self.tiles[ci][:, r - ci * L2_CHUNK, :], ci

    def dep(self, inst, ci):
        # PE executes in order: only the FIRST consuming matmul of a chunk
        # needs the sync edge; later PE instructions are ordered behind it.
        if ci in self.dep_done:
            return
        self.dep_done.add(ci)
        add_dep_helper(_inst(inst), _inst(self.wi[ci]), sync=True,
                       reason=f"gather consume {self.name}:{ci}")


class _Stream:
    """Sequential chunked load of a host-built [P, n, F] DRAM stream."""

    PREFETCH = int(_os.environ.get("K_SPF", "3"))

    def __init__(self, nc, pool, src_d, n_runs, chunk, tag):
        self.nc, self.pool, self.src_d, self.n_runs = nc, pool, src_d, n_runs
        self.chunk, self.tag = chunk, tag
        self.n_chunks = cdiv(n_runs, chunk)
        self.tiles = {}
        self.issued = -1

    def _issue(self, ci):
        nruns = min(self.chunk, self.n_runs - ci * self.chunk)
        t = self.pool.tile([P, self.chunk, F], mybir.dt.bfloat16,
                           tag=self.tag, name=f"{self.tag}_{ci}")
        self.nc.sync.dma_start(
            out=t[:, :nruns, :],
            in_=self.src_d[:, ci * self.chunk:ci * self.chunk + nruns, :])
        self.tiles[ci] = t
        self.issued = ci

    def rhs(self, r):
        ci = r // self.chunk
        while self.issued < min(ci + self.PREFETCH, self.n_chunks - 1):
            self._issue(self.issued + 1)
        self.tiles.pop(ci - 2, None)
        return self.tiles[ci][:, r - ci * self.chunk, :]


def _build(struct, trivial, for_sim=False):
    R1, RA, RB = struct["R1"], struct["RA"], struct["RB"]
    rs1, re1 = struct["rs1"], struct["re1"]
    rs2, re2 = struct["rs2"], struct["re2"]
    n_ops1, n_ops2 = struct["n_ops1"], struct["n_ops2"]

    nc = bacc.Bacc("TRN2", target_bir_lowering=False, debug=False,
                   num_devices=1 if for_sim else NCORES)
    f32 = mybir.dt.float32
    bfl = mybir.dt.bfloat16

    # ---- I/O
    g0_d = nc.dram_tensor("g0", [P, R1, F], bfl, kind="ExternalInput")
    xself_d = nc.dram_tensor("xself", [P, NB, F], bfl, kind="ExternalInput")
    ns1, ns2 = struct["ns1"], struct["ns2"]
    s1s_d = nc.dram_tensor("s1s", [P, max(ns1, 1), P], bfl, kind="ExternalInput")
    s2s_d = nc.dram_tensor("s2s", [P, max(ns2, 1), P], bfl, kind="ExternalInput")
    dr1_d = nc.dram_tensor("dr1", [P, n_ops1], f32, kind="ExternalInput")
    dr2_d = nc.dram_tensor("dr2", [P, n_ops2], f32, kind="ExternalInput")
    idxA_d = nc.dram_tensor("idxA", [P, max(RA * 8, 1)], mybir.dt.int16, kind="ExternalInput")
    idxB_d = nc.dram_tensor("idxB", [P, max(RB * 8, 1)], mybir.dt.int16, kind="ExternalInput")
    cnts_d = nc.dram_tensor("cnts", [P, NB], f32, kind="ExternalInput")
    w_d = [nc.dram_tensor(f"W{l+1}", [F, F], bfl, kind="ExternalInput") for l in range(2)]
    identb_d = nc.dram_tensor("identb", [P, P], bfl, kind="ExternalInput")
    colio_d = nc.dram_tensor("colio", [P, P], bfl, kind="ExternalInput")
    aff_d = []
    if not trivial:
        for l in range(2):
            aff_d.append({k: nc.dram_tensor(f"{k}{l+1}", [P, F], f32, kind="ExternalInput")
                          for k in ("bB", "gB", "beB")})
    out_d = nc.dram_tensor("out", [PPAD, F], f32, kind="ExternalOutput")

    import contextlib
    with tile.TileContext(nc) as tc, contextlib.ExitStack() as st:
        gsems = [st.enter_context(nc.semaphore(f"gsem{i}")) for i in range(N_GSEMS)]
        sem_counts = [0] * N_GSEMS
        pers = st.enter_context(tc.tile_pool(name="pers", bufs=1))
        strm = st.enter_context(tc.tile_pool(name="strm", bufs=int(_os.environ.get("K_STRMB", "5"))))
        gpool = st.enter_context(tc.tile_pool(name="gpool", bufs=GPOOL_BUFS))
        spool = st.enter_context(tc.tile_pool(name="spool", bufs=int(_os.environ.get("K_SBUFS", "24"))))
        sbpool = st.enter_context(tc.tile_pool(name="sbpool", bufs=4))
        tpool = st.enter_context(tc.tile_pool(name="tpool", bufs=int(_os.environ.get("K_TPB", "6"))))
        vpool = st.enter_context(tc.tile_pool(name="vpool", bufs=int(_os.environ.get("K_VPB", "12"))))
        gr_pool = st.enter_context(tc.tile_pool(name="grp", bufs=3))
        pre_ps = st.enter_context(tc.tile_pool(name="pre_ps", bufs=int(_os.environ.get("K_PREB", "2")), space="PSUM"))
        ag_ps = st.enter_context(tc.tile_pool(name="ag_ps", bufs=GRP, space="PSUM"))
        tr_ps = st.enter_context(tc.tile_pool(name="tr_ps", bufs=int(_os.environ.get("K_TRB", "1")), space="PSUM"))
        ph_ps = st.enter_context(tc.tile_pool(name="ph_ps", bufs=int(_os.environ.get("K_PHB", "1")), space="PSUM"))
        dram = st.enter_context(tc.tile_pool(name="dram", bufs=1, space="DRAM"))

        nc.gpsimd.load_library(_mlp_lib)

        # ---- persistent loads
        def load(name, dten, shape, dt=None):
            t = pers.tile(shape, dt or f32, name=name)
            nc.sync.dma_start(out=t[:], in_=dten[:])
            return t

        cnts = load("cnts_t", cnts_d, [P, NB])
        xself = load("xself_t", xself_d, [P, NB, F], bfl)
        w_t = [load(f"w{l}_t", w_d[l], [F, F], bfl) for l in range(2)]
        identb = load("identb_t", identb_d, [P, P], bfl)
        colio = load("colio_t", colio_d, [P, P], bfl)
        dr1 = load("dr1_t", dr1_d, [P, n_ops1])
        dr2 = load("dr2_t", dr2_d, [P, n_ops2])
        idxA = pers.tile([P, max(RA * 8, 1)], mybir.dt.int16, name="idxA_t")
        nc.sync.dma_start(out=idxA[:], in_=idxA_d[:])
        idxB = pers.tile([P, max(RB * 8, 1)], mybir.dt.int16, name="idxB_t")
        nc.sync.dma_start(out=idxB[:], in_=idxB_d[:])
        aff = []
        if not trivial:
            for l in range(2):
                aff.append({k: load(f"{k}{l}_t", d, [P, F]) for k, d in aff_d[l].items()})

        hs_self = pers.tile([P, NB, F], bfl, name="hs_self")
        s1 = pers.tile([P, NB, F], bfl, name="s1")

        # dinv = 1/sqrt(counts + 1)  (for the s1 fold only)
        dsq = pers.tile([P, NB], f32, name="dsq")
        nc.scalar.activation(out=dsq[:], in_=cnts[:],
                             func=mybir.ActivationFunctionType.Sqrt, bias=1.0)
        dinv = pers.tile([P, NB], f32, name="dinv")
        nc.vector.reciprocal(out=dinv[:], in_=dsq[:])

        eps_t = pers.tile([P, 1], f32, name="eps_t")
        nc.vector.memset(eps_t[:], LN_EPS)

        hs2_loc = dram.tile([PPAD, F], bfl, name="hs2_loc")
        hs2_full = dram.tile([NPAD, F], bfl, name="hs2_full",
                             addr_space="Shared")

        op_ctr = [0, 0]   # dr column counters per layer
        s_ctr = [0, 0]    # streamed-S counters per layer
        s_streams = [
            _Stream(nc, strm, s1s_d, ns1, SS_CHUNK, "s1chunk") if ns1 else None,
            _Stream(nc, strm, s2s_d, ns2, SS_CHUNK, "s2chunk") if ns2 else None,
        ]

        def build_S(l, dr_t):
            col = op_ctr[l]
            op_ctr[l] += 1
            if _s_streamed(l, col):
                i = s_ctr[l]
                s_ctr[l] += 1
                return s_streams[l].rhs(i)
            S = spool.tile([P, P], bfl, tag="S", name=f"S_{l}_{col}")
            if _s_on_pool(l, col):
                nc.gpsimd.tensor_scalar(out=S[:], in0=colio[:],
                                        scalar1=dr_t[:, col:col + 1], scalar2=None,
                                        op0=mybir.AluOpType.is_equal)
            elif _s_on_act(l, col):
                t = sbpool.tile([P, P], bfl, tag="St", name=f"St_{l}_{col}")
                nc.scalar.activation(out=t[:], in_=colio[:],
                                     func=mybir.ActivationFunctionType.Abs,
                                     bias=dr_t[:, col:col + 1], scale=-1.0)
                nc.scalar.activation(out=S[:], in_=t[:],
                                     func=mybir.ActivationFunctionType.Relu,
                                     bias=1.0, scale=-1.0)
            else:
                nc.vector.tensor_scalar(out=S[:], in0=colio[:],
                                        scalar1=dr_t[:, col:col + 1], scalar2=None,
                                        op0=mybir.AluOpType.is_equal)
            return S[:]

        def make_S_block(l, dr_t, n):
            """S APs for the next n ops of layer l. SBATCH mode: one DVE
            tensor_tensor builds all n one-hots (one dep for the whole
            matmul chain); else per-op tensor_scalar / stream."""
            c0 = op_ctr[l]
            if SBATCH[l] and n > 0:
                assert n <= SBATCH_MAX
                op_ctr[l] += n
                SB = sbpool.tile([P, SBATCH_MAX, P], bfl, tag="SB",
                                name=f"SB_{l}_{c0}")
                nc.vector.tensor_tensor(
                    out=SB[:, :n, :],
                    in0=dr_t[:, c0:c0 + n, None].to_broadcast([P, n, P]),
                    in1=colio[:, None, :].to_broadcast([P, n, P]),
                    op=mybir.AluOpType.is_equal)
                return [SB[:, i, :] for i in range(n)]
            return [build_S(l, dr_t) for _ in range(n)]

        def copy_t(out_ap, in_ap):
            if COPY_ENGINE == "act":
                nc.scalar.activation(out=out_ap, in_=in_ap,
                                     func=mybir.ActivationFunctionType.Copy)
            elif COPY_ENGINE == "gps":
                nc.gpsimd.tensor_copy(out=out_ap, in_=in_ap)
            else:
                nc.vector.tensor_copy(out=out_ap, in_=in_ap)

        # ---- epilogue machinery -------------------------------------------
        # group context: per-block stats collected, sqrt batched per group
        def pre_affine(l, b, psum):
            """Nontrivial-b path: materialize pre = dinv*psum + b before LN
            (the b bias breaks LN's row-scale invariance)."""
            pre = tpool.tile([P, F], f32, tag="pre", name=f"pre_{l}_{b}")
            nc.scalar.activation(out=pre[:], in_=psum[:],
                                 func=mybir.ActivationFunctionType.Copy,
                                 scale=dinv[:, b:b + 1])
            pre2 = tpool.tile([P, F], f32, tag="pre2", name=f"pre2_{l}_{b}")
            nc.vector.tensor_tensor(out=pre2[:], in0=pre[:], in1=aff[l]["bB"][:],
                                    op=mybir.AluOpType.add)
            return pre2

        def stats(l, b, psum, grp_rc, gi):
            """bn_stats/aggr on psum; write 1/(var+eps) into grp_rc col gi;
            returns the [P,2] mean/var tile."""
            st6 = vpool.tile([P, 6], f32, tag="st6", name=f"st6_{l}_{b}")
            nc.vector.bn_stats(out=st6[:], in_=psum[:])
            mv = vpool.tile([P, 2], f32, tag="mv", name=f"mv_{l}_{b}")
            nc.vector.bn_aggr(out=mv[:], in_=st6[:])
            if _os.environ.get("K_RECIP", "dve") == "act":
                nc.scalar.activation(out=grp_rc[:, gi:gi + 1], in_=mv[:, 1:2],
                                     func=mybir.ActivationFunctionType.Reciprocal,
                                     bias=eps_t[:, 0:1])
            else:
                vv = vpool.tile([P, 1], f32, tag="vv", name=f"vv_{l}_{b}")
                nc.vector.tensor_tensor(out=vv[:], in0=mv[:, 1:2], in1=eps_t[:],
                                        op=mybir.AluOpType.add)
                nc.vector.reciprocal(out=grp_rc[:, gi:gi + 1], in_=vv[:])
            return mv

        def final(l, b, psum, mv, grp_rv, gi):
            """out = relu((psum - mu) * rv [* dinv_b for l=0]); l=0 writes s1
            block (bf16), l=1 writes DRAM out rows (f32)."""
            if l == 0 and trivial:
                rv2 = vpool.tile([P, 1], f32, tag="rv2", name=f"rv2_{b}")
                nc.vector.tensor_tensor(out=rv2[:], in0=grp_rv[:, gi:gi + 1],
                                        in1=dinv[:, b:b + 1],
                                        op=mybir.AluOpType.mult)
                rv_ap = rv2[:, 0:1]
            else:
                rv_ap = grp_rv[:, gi:gi + 1]
            bias2 = vpool.tile([P, 1], f32, tag="b2", name=f"b2_{l}_{b}")
            nc.vector.scalar_tensor_tensor(
                out=bias2[:], in0=mv[:, 0:1], scalar=-1.0, in1=rv_ap,
                op0=mybir.AluOpType.mult, op1=mybir.AluOpType.mult)
            if not trivial:
                # generic (slow, unused for harness inputs): psum here is the
                # pre_affine output (dinv*agg + b); y = xn*g + be,
                # l=0: s1 = relu(y * dinv), l=1: out = relu(y)
                xn = tpool.tile([P, F], f32, tag="xn", name=f"xn_{l}_{b}")
                nc.scalar.activation(out=xn[:], in_=psum[:],
                                     func=mybir.ActivationFunctionType.Identity,
                                     bias=bias2[:, 0:1], scale=rv_ap)
                y = tpool.tile([P, F], f32, tag="y", name=f"y_{l}_{b}")
                nc.vector.tensor_tensor(out=y[:], in0=xn[:], in1=aff[l]["gB"][:],
                                        op=mybir.AluOpType.mult)
                y2 = tpool.tile([P, F], f32, tag="y2", name=f"y2_{l}_{b}")
                nc.vector.tensor_tensor(out=y2[:], in0=y[:], in1=aff[l]["beB"][:],
                                        op=mybir.AluOpType.add)
                if l == 0:
                    ys = tpool.tile([P, F], f32, tag="ys", name=f"ys_{b}")
                    nc.vector.tensor_scalar(out=ys[:], in0=y2[:],
                                            scalar1=dinv[:, b:b + 1], scalar2=None,
                                            op0=mybir.AluOpType.mult)
                    nc.scalar.activation(out=s1[:, b, :], in_=ys[:],
                                         func=mybir.ActivationFunctionType.Relu)
                else:
                    ot = tpool.tile([P, F], f32, tag="ot", name=f"ot_{b}")
                    nc.scalar.activation(out=ot[:], in_=y2[:],
                                         func=mybir.ActivationFunctionType.Relu)
                    nc.sync.dma_start(
                        out=out_d[:].rearrange("(n p) f -> p n f", p=P)[:, b, :],
                        in_=ot[:])
                return
            if l == 0:
                nc.scalar.activation(out=s1[:, b, :], in_=psum[:],
                                     func=mybir.ActivationFunctionType.Relu,
                                     bias=bias2[:, 0:1], scale=rv_ap)
            else:
                ot = tpool.tile([P, F], f32, tag="ot", name=f"ot_{b}")
                nc.scalar.activation(out=ot[:], in_=psum[:],
                                     func=mybir.ActivationFunctionType.Relu,
                                     bias=bias2[:, 0:1], scale=rv_ap)
                nc.sync.dma_start(
                    out=out_d[:].rearrange("(n p) f -> p n f", p=P)[:, b, :],
                    in_=ot[:])

        def phase_a2_block(b):
            """hs2 block: transpose s1_b, hs2 = s1 @ W2 (via lhsT = s1^T),
            keep bf16 copy in hs_self and DMA (grouped) to hs2_loc."""
            pt = tr_ps.tile([P, P], bfl, tag="pt", name=f"pt_{b}")
            nc.tensor.transpose(out=pt[:], in_=s1[:, b, :], identity=identb[:])
            lt = tpool.tile([P, P], bfl, tag="lt", name=f"lt_{b}")
            copy_t(lt[:], pt[:])
            ph = ph_ps.tile([P, F], f32, tag="ph", name=f"ph_{b}")
            nc.tensor.matmul(out=ph[:], lhsT=lt[:], rhs=w_t[1][:],
                             start=True, stop=True)
            copy_t(hs_self[:, b, :], ph[:])
            W = int(_os.environ.get("K_HSW", "4"))
            if b % W == W - 1 or b == NB - 1:
                b0 = (b // W) * W
                nc.sync.dma_start(
                    out=hs2_loc[:].rearrange("(n p) f -> p n f", p=P)[:, b0:b + 1, :],
                    in_=hs_self[:, b0:b + 1, :])

        # =================== LAYER 1 ===================
        stream = _Stream(nc, strm, g0_d, R1, L1_CHUNK, "g0chunk")

        def l1_block_agg(b):
            """Aggregate Xs rows (transposed): preT[f,j] += G0^T S, then
            out1 = preT^T @ W1."""
            preT = pre_ps.tile([P, P], f32, tag="preT", name=f"preT_{b}")
            nmm = re1[b] - rs1[b]
            nc.tensor.matmul(out=preT[:], lhsT=xself[:, b, :], rhs=identb[:],
                             start=True, stop=(nmm == 0))
            Sb = make_S_block(0, dr1, nmm)
            for k, r in enumerate(range(rs1[b], re1[b])):
                rhs_g = stream.rhs(r)
                nc.tensor.matmul(out=preT[:], lhsT=rhs_g, rhs=Sb[k],
                                 start=False, stop=(k == nmm - 1))
            ltT = tpool.tile([P, P], bfl, tag="ltT", name=f"ltT_{b}")
            copy_t(ltT[:], preT[:])
            ops = ag_ps.tile([P, F], f32, tag="agg", name=f"agg1_{b}")
            nc.tensor.matmul(out=ops[:], lhsT=ltT[:], rhs=w_t[0][:],
                             start=True, stop=True)
            return ops

        for g0 in range(0, NB, GRP):
            blocks = list(range(g0, min(g0 + GRP, NB)))
            grp_rc = gr_pool.tile([P, GRP], f32, tag="grc", name=f"grc1_{g0}")
            info = []
            for gi, b in enumerate(blocks):
                psum = l1_block_agg(b)
                if not trivial:
                    psum = pre_affine(0, b, psum)
                mv = stats(0, b, psum, grp_rc, gi)
                info.append((b, psum, mv))
            grp_rv = gr_pool.tile([P, GRP], f32, tag="grv", name=f"grv1_{g0}")
            nc.scalar.activation(out=grp_rv[:, :len(blocks)],
                                 in_=grp_rc[:, :len(blocks)],
                                 func=mybir.ActivationFunctionType.Sqrt)
            for gi, (b, psum, mv) in enumerate(info):
                final(0, b, psum, mv, grp_rv, gi)
                phase_a2_block(b)

        # =================== AllGather (the only collective) ===============
        if for_sim:
            nc.sync.dma_start(out=hs2_full[0:PPAD, :], in_=hs2_loc[:])
        else:
            nc.gpsimd.collective_compute(
                "AllGather", mybir.AluOpType.bypass,
                ins=[hs2_loc[:]], outs=[hs2_full[:]],
                replica_groups=[list(range(NCORES))])

        # =================== LAYER 2 ===================
        gA = _Gather(nc, tc, gpool, gsems, sem_counts, idxA,
                     hs2_full[0:SPLIT, :], RA, "A")
        gB = _Gather(nc, tc, gpool, gsems, sem_counts, idxB,
                     hs2_full[SPLIT:NPAD, :], RB, "B")

        def l2_block_agg(b):
            psum = ag_ps.tile([P, F], f32, tag="agg", name=f"agg2_{b}")
            nmm = sum(re2[h][b] - rs2[h][b] for h in range(2))
            nc.tensor.matmul(out=psum[:], lhsT=identb[:], rhs=hs_self[:, b, :],
                             start=True, stop=(nmm == 0))
            Sb = make_S_block(1, dr2, nmm)
            k = 0
            gs = (gA, gB)
            for h, r in _l2_op_order(rs2, re2, b):
                g = gs[h]
                rhs_g, ci = g.rhs(r)
                S = Sb[k]
                k += 1
                mm = nc.tensor.matmul(out=psum[:], lhsT=S, rhs=rhs_g,
                                      start=False, stop=(k == nmm))
                g.dep(mm, ci)
            return psum

        for g0 in range(0, NB, GRP):
            blocks = list(range(g0, min(g0 + GRP, NB)))
            grp_rc = gr_pool.tile([P, GRP], f32, tag="grc", name=f"grc2_{g0}")
            info = []
            for gi, b in enumerate(blocks):
                psum = l2_block_agg(b)
                if not trivial:
                    psum = pre_affine(1, b, psum)
                mv = stats(1, b, psum, grp_rc, gi)
                info.append((b, psum, mv))
            grp_rv = gr_pool.tile([P, GRP], f32, tag="grv", name=f"grv2_{g0}")
            nc.scalar.activation(out=grp_rv[:, :len(blocks)],
                                 in_=grp_rc[:, :len(blocks)],
                                 func=mybir.ActivationFunctionType.Sqrt)
            for gi, (b, psum, mv) in enumerate(info):
                final(1, b, psum, mv, grp_rv, gi)

        assert op_ctr[0] == n_ops1 and op_ctr[1] == n_ops2, (op_ctr, n_ops1, n_ops2)

    nc.compile()
    return nc


# ---------------------------------------------------------------- entry point
LAST_EXEC_NS = None
_LAST_STRUCT = None


def estimate_ns():
    """Single-core TimelineSim cost-model estimate of the compiled program
    (collective replaced by a local copy). Not a hardware measurement."""
    global LAST_EXEC_NS
    if _LAST_STRUCT is None:
        return None
    from trails.perfetto import LazyPerfetto
    for _m in ("enable_explicit_ordering", "reserve_process_order", "add_counter"):
        if not hasattr(LazyPerfetto, _m):
            setattr(LazyPerfetto, _m, lambda self, *a, **k: None)
    from concourse.timeline_sim import TimelineSim
    nc = _build(_LAST_STRUCT, trivial=True, for_sim=True)
    sim = TimelineSim(nc, trace=False)
    sim.simulate()
    LAST_EXEC_NS = int(sim.time)
    return LAST_EXEC_NS


def kernel(x, edge_index, W1, b1, g1, be1, W2, b2, g2, be2):
    global _LAST_STRUCT
    x = np.asarray(x)
    in_maps, struct = _prep(x, edge_index)
    _LAST_STRUCT = struct

    trivial = all(
        (np.all(np.asarray(b) == 0.0) and np.all(np.asarray(g) == 1.0)
         and np.all(np.asarray(be) == 0.0))
        for b, g, be in ((b1, g1, be1), (b2, g2, be2)))

    nc = _build(struct, trivial)

    ident = np.eye(P, dtype=np.float32)
    colio = np.tile(np.arange(P, dtype=np.float32)[None, :], (P, 1))
    shared = {
        "W1": np.asarray(W1, dtype=np.float32).astype(bf16),
        "W2": np.asarray(W2, dtype=np.float32).astype(bf16),
        "identb": ident.astype(bf16),
        "colio": colio.astype(bf16),
    }
    if not trivial:
        for l, (b, g, be) in enumerate(((b1, g1, be1), (b2, g2, be2))):
            shared[f"bB{l+1}"] = np.tile(np.asarray(b, np.float32)[None, :], (P, 1))
            shared[f"gB{l+1}"] = np.tile(np.asarray(g, np.float32)[None, :], (P, 1))
            shared[f"beB{l+1}"] = np.tile(np.asarray(be, np.float32)[None, :], (P, 1))
    for m in in_maps:
        m.update(shared)

    res = run_bass_kernel_spmd(nc, in_maps, core_ids=list(range(NCORES)))
    out = np.concatenate([res.results[c]["out"][:PART] for c in range(NCORES)], axis=0)
    return out.astype(np.float32)

